# revision 1
# baseline (speedup 1.0000x reference)
"""FAGCN (2x FAConv) distributed Bass kernel for 8 Trainium2 NeuronCores.

Sharding: 12500 nodes/core (padded 12544); edges dst-sharded. Per layer the
cores AllGather a bf16 table [100352, 128] = [h(64)|hl|pad] and gather source
rows per edge with dma_gather (int16 idx over 4x32768-row windows, 4 SWDGE
queues). Per-edge coef tanh(hl_src+hr_dst)*sigmoid(ew) and segment sums run
on ACT/DVE in a node-partition supertile layout ([128 nodes x padded slot
cols], slots grouped by window; nodes packed by per-window degree lexsort).
t1/t2 matmuls on TensorE with host-pretransposed weights (bias folded).
"""
import sys
sys.path.insert(0, "/opt/trn_rl_repo")
import numpy as np

# problem constants (hardcoded per harness contract)
N = 100000
E = 1600000
F_IN = 500
D = 64
C = 40
EPS = 0.3
NC = 8
NPC = N // NC            # 12500
NT = 98                  # supertiles per core
NPAD = NT * 128          # 12544
TROWS = NC * NPAD        # 100352 table rows
WIN = 32768
NWIN = (TROWS + WIN - 1) // WIN   # 4
FA = 504                 # padded t1 contraction (500 feat + bias + pad)
KC = 126                 # contraction chunk (4 x 126)
TW = 128                 # bf16 table row width (256B)


# ---------------------------------------------------------------- host prep
def _pack(dw):
    # group nodes by dominant window, then peak/total degree desc; then
    # align supertiles across cores by K-profile so harmonized maxima stay
    # close to per-core maxima.
    am = dw.argmax(1)
    o = np.lexsort((-dw.sum(1), -dw.max(1), am))
    Kc = dw[o].reshape(NT, 128, NWIN).max(axis=1)
    pt = np.lexsort(tuple(-Kc[:, w] for w in range(NWIN - 1, -1, -1)))
    return o.reshape(NT, 128)[pt].reshape(-1)


def prep(x, edge_index, edge_weight_train):
    src = np.asarray(edge_index[0]).astype(np.int64)
    dst = np.asarray(edge_index[1]).astype(np.int64)
    ew = np.asarray(edge_weight_train, np.float32)
    x = np.asarray(x, np.float32)
    core_of_dst = dst // NPC

    # pass 1: provisional packing with identity-layout source windows
    row0 = (src // NPC) * NPAD + (src % NPC)
    win0 = row0 // WIN
    orders = []
    for c in range(NC):
        m = core_of_dst == c
        dw = np.zeros((NPAD, NWIN), np.int64)
        np.add.at(dw, (dst[m] - c * NPC, win0[m]), 1)
        orders.append(_pack(dw))
    # true table rows under this packing (no second repack: windows shift
    # only near 32768 boundaries; degrees recomputed exactly below)
    posmap = np.empty(N, np.int64)
    for c in range(NC):
        inv = np.empty(NPAD, np.int64)
        inv[orders[c]] = np.arange(NPAD)
        posmap[c * NPC:(c + 1) * NPC] = c * NPAD + inv[:NPC]
    srow = posmap[src]
    swin = srow // WIN

    # exact per-(packed dst node, window) degrees; K harmonized across cores
    K = np.zeros((NT, NWIN), np.int64)
    percore = []
    for c in range(NC):
        m = core_of_dst == c
        inv = np.empty(NPAD, np.int64)
        inv[orders[c]] = np.arange(NPAD)
        dpos = inv[dst[m] - c * NPC]
        dw = np.zeros((NPAD, NWIN), np.int64)
        np.add.at(dw, (dpos, swin[m]), 1)
        K = np.maximum(K, dw.reshape(NT, 128, NWIN).max(axis=1))
        percore.append((m, dpos))
    off = np.zeros((NT, NWIN), np.int64)
    for t in range(NT):
        o = 0
        for w in range(NWIN):
            off[t, w] = o
            o += K[t, w]
    skt = K.sum(axis=1)
    tbase = np.concatenate([[0], np.cumsum(skt)]).astype(np.int64)
    totslots = int(tbase[-1])

    in_maps = []
    for c in range(NC):
        m, dpos = percore[c]
        sw = swin[m]
        sr = srow[m] - sw * WIN
        ewc = ew[m]
        order = np.lexsort((sr, sw, dpos))
        ds, ws, rs, es = dpos[order], sw[order], sr[order], ewc[order]
        grp = ds * NWIN + ws
        first = np.r_[True, grp[1:] != grp[:-1]]
        idxf = np.arange(len(grp))
        start = np.maximum.accumulate(np.where(first, idxf, 0))
        rank = idxf - start
        tt, pp = ds // 128, ds % 128
        slot = tbase[tt] + off[tt, ws] + rank

        ew_arr = np.zeros((128, totslots), np.float32)
        ew_arr[pp, slot] = es
        flat = np.zeros((totslots, 128), np.int64)
        flat[slot, pp] = rs
        idx_arr = np.zeros((128, 8 * totslots), np.int16)
        for t in range(NT):
            for w in range(NWIN):
                kw = int(K[t, w])
                if kw == 0:
                    continue
                cb = int(tbase[t] + off[t, w])
                fl = flat[cb:cb + kw, :].reshape(-1)       # i = d*128+p
                wrapped = fl.reshape(8 * kw, 16).T          # [16, 8k]
                idx_arr[:, 8 * cb:8 * (cb + kw)] = np.tile(
                    wrapped, (8, 1)).astype(np.int16)

        xp = np.zeros((NPAD, F_IN), np.float32)
        onode = orders[c]
        real = onode < NPC
        xp[real] = x[c * NPC + onode[real]]
        xT = np.zeros((FA, NPAD), np.float32)
        xT[:F_IN] = xp.T
        xT[F_IN] = 1.0
        in_maps.append({"xT": xT, "idx16": idx_arr, "ewv": ew_arr})
    return in_maps, orders, K, off, tbase, totslots


def make_weight_maps(t1_w, t1_b, att_l1, att_r1, att_l2, att_r2, t2_w, t2_b):
    t1f = np.zeros((FA, D), np.float32)
    t1f[:F_IN] = np.asarray(t1_w, np.float32).T
    t1f[F_IN] = np.asarray(t1_b, np.float32)
    t1wT = t1f.reshape(4, KC, D).transpose(1, 0, 2).reshape(KC, 4 * D)
    t2wT = np.zeros((D + 1, C), np.float32)
    t2wT[:D] = np.asarray(t2_w, np.float32).T
    t2wT[D] = np.asarray(t2_b, np.float32)
    rep = lambda v: np.tile(np.asarray(v, np.float32)[None, :], (128, 1))
    return {"t1wT": t1wT, "t2wT": t2wT,
            "attl1": rep(att_l1), "attr1": rep(att_r1),
            "attl2": rep(att_l2), "attr2": rep(att_r2)}


# ---------------------------------------------------------------- device prog
def build_program(K, off, tbase, totslots):
    import concourse.bass as bass
    import concourse.bacc as bacc
    import concourse.mybir as mybir
    import concourse.tile as tile
    from concourse.masks import make_identity
    f32, bf16, i16 = (mybir.dt.float32, mybir.dt.bfloat16, mybir.dt.int16)

    nc = bacc.Bacc("TRN2", target_bir_lowering=False, debug=False,
                   num_devices=NC, num_swdge_queues=4,
                   dynamic_dma_scratch_size=32768)
    xT = nc.dram_tensor("xT", [FA, NPAD], f32, kind="ExternalInput")
    t1wT = nc.dram_tensor("t1wT", [KC, 4 * D], f32, kind="ExternalInput")
    t2wT = nc.dram_tensor("t2wT", [D + 1, C], f32, kind="ExternalInput")
    attl1 = nc.dram_tensor("attl1", [128, D], f32, kind="ExternalInput")
    attr1 = nc.dram_tensor("attr1", [128, D], f32, kind="ExternalInput")
    attl2 = nc.dram_tensor("attl2", [128, D], f32, kind="ExternalInput")
    attr2 = nc.dram_tensor("attr2", [128, D], f32, kind="ExternalInput")
    idx16 = nc.dram_tensor("idx16", [128, 8 * totslots], i16,
                           kind="ExternalInput")
    ewv = nc.dram_tensor("ewv", [128, totslots], f32, kind="ExternalInput")
    out = nc.dram_tensor("out", [NPAD, C], f32, kind="ExternalOutput")

    sk_max = int(max(sum(K[t]) for t in range(NT)))

    def seg_bcast(ap2d, segs, width):
        # [128, segs] AP -> [128, segs, width] AP (inner broadcast)
        return bass.AP(ap2d.tensor, ap2d.offset,
                       [ap2d.ap[0], (ap2d.ap[-1][0], segs), (0, width)])

    def rep_mid(ap2d, segs, width):
        # [128, width] AP -> [128, segs, width] AP (middle repeat)
        return bass.AP(ap2d.tensor, ap2d.offset,
                       [ap2d.ap[0], (0, segs), (1, width)])

    with tile.TileContext(nc) as tc:
        with tc.tile_pool(name="sbuf", bufs=1) as sbp, \
             tc.tile_pool(name="dram", bufs=1, space="DRAM") as dram, \
             tc.tile_pool(name="psum", bufs=1, space="PSUM") as psp:

            h0_sb = sbp.tile([128, NT * D], f32)
            h1_sb = sbp.tile([128, NT * D], f32)
            hl_sb = sbp.tile([128, NT], f32)
            hr_sb = sbp.tile([128, NT], f32)
            ew_sb = sbp.tile([128, totslots], f32)
            w1_sb = sbp.tile([KC, 4 * D], f32)
            w2_sb = sbp.tile([D + 1, C], f32)
            al1_sb = sbp.tile([128, D], f32)
            ar1_sb = sbp.tile([128, D], f32)
            al2_sb = sbp.tile([128, D], f32)
            ar2_sb = sbp.tile([128, D], f32)
            ident = sbp.tile([128, 128], f32)
            logit_sb = sbp.tile([128, NT * C], f32)

            tab1 = dram.tile([TROWS, TW], bf16, addr_space="Shared",
                             name="tab1")
            tab2 = dram.tile([TROWS, TW], bf16, addr_space="Shared",
                             name="tab2")
            ltab1 = dram.tile([TROWS, TW], bf16, name="ltab1")
            ltab2 = dram.tile([TROWS, TW], bf16, name="ltab2")
            aug_loc1 = dram.tile([NPAD, TW], bf16, name="aug_loc1")
            aug_loc2 = dram.tile([NPAD, TW], bf16, name="aug_loc2")

            make_identity(nc, ident[:])
            nc.sync.dma_start(out=w1_sb[:], in_=t1wT[:, :])
            nc.sync.dma_start(out=w2_sb[:], in_=t2wT[:, :])
            nc.sync.dma_start(out=al1_sb[:], in_=attl1[:, :])
            nc.sync.dma_start(out=ar1_sb[:], in_=attr1[:, :])
            nc.sync.dma_start(out=al2_sb[:], in_=attl2[:, :])
            nc.sync.dma_start(out=ar2_sb[:], in_=attr2[:, :])

            with nc.named_scope("ew_prep"):
                ewr = sbp.tile([128, totslots], f32, tag="xstage", bufs=2)
                nc.sync.dma_start(out=ewr[:], in_=ewv[:, :])
                nc.scalar.activation(ew_sb[:], ewr[:],
                                     mybir.ActivationFunctionType.Sigmoid)
                nc.vector.tensor_scalar(out=ewr[:], in0=ewr[:], scalar1=0.0,
                                        scalar2=None,
                                        op0=mybir.AluOpType.not_equal)
                nc.vector.tensor_mul(out=ew_sb[:], in0=ew_sb[:], in1=ewr[:])

            # phase 1: h0 = relu(x @ t1_w.T + b)
            with nc.named_scope("t1_matmul"):
                CH = 4
                for blk in range((NT + CH - 1) // CH):
                    n0 = blk * CH * 128
                    nch = min(CH, NT - blk * CH)
                    pt = [psp.tile([128, D], f32, tag=f"p1_{t}", bufs=1,
                                   name=f"p1_{blk}_{t}") for t in range(nch)]
                    for cc in range(4):
                        xc = sbp.tile([KC, CH * 128], f32, tag="xstage",
                                      bufs=2)
                        nc.sync.dma_start(
                            out=xc[:, :nch * 128],
                            in_=xT[cc * KC:(cc + 1) * KC, n0:n0 + nch * 128])
                        for t in range(nch):
                            nc.tensor.matmul(
                                pt[t][:], lhsT=xc[:, t * 128:(t + 1) * 128],
                                rhs=w1_sb[:, cc * D:(cc + 1) * D],
                                start=(cc == 0), stop=(cc == 3))
                    for t in range(nch):
                        gt = blk * CH + t
                        nc.scalar.activation(
                            h0_sb[:, gt * D:(gt + 1) * D], pt[t][:],
                            mybir.ActivationFunctionType.Relu)

            def att_dots(h_sb, att_sb, out_col):
                CHT = 14
                for b in range(NT // CHT):
                    tmp = sbp.tile([128, CHT * D], f32, tag="adtmp", bufs=2)
                    sl = h_sb[:, b * CHT * D:(b + 1) * CHT * D]
                    nc.vector.tensor_tensor(
                        out=tmp[:], in0=sl, in1=rep_mid(att_sb[:, :], CHT, D),
                        op=mybir.AluOpType.mult)
                    nc.vector.tensor_reduce(
                        out=out_col[:, b * CHT:(b + 1) * CHT],
                        in_=tmp[:].rearrange("p (t d) -> p t d", t=CHT, d=D),
                        axis=mybir.AxisListType.X, op=mybir.AluOpType.add)

            def build_aug(h_sb, al_sb, aug_loc, tabdst, ltabdst, scope):
                with nc.named_scope(scope):
                    att_dots(h_sb, al_sb, hl_sb)
                    for t in range(NT):
                        aug = sbp.tile([128, TW], bf16, tag="augt", bufs=3)
                        nc.vector.tensor_copy(
                            out=aug[:, :D], in_=h_sb[:, t * D:(t + 1) * D])
                        nc.vector.tensor_copy(out=aug[:, D:D + 1],
                                              in_=hl_sb[:, t:t + 1])
                        nc.sync.dma_start(
                            out=aug_loc[t * 128:(t + 1) * 128, :],
                            in_=aug[:])
                    nc.gpsimd.collective_compute(
                        "AllGather", mybir.AluOpType.bypass,
                        replica_groups=[list(range(NC))],
                        ins=[aug_loc[:, :]], outs=[tabdst[:, :]])
                    nc.sync.dma_start(out=ltabdst[:, :], in_=tabdst[:, :])

            qrr = [0]

            def layer(tabsrc, src_sb, dst_sb, ar_sb, scope):
                with nc.named_scope(scope):
                    att_dots(src_sb, ar_sb, hr_sb)
                    for t in range(NT):
                        sk = int(sum(K[t]))
                        base = int(tbase[t])
                        if sk == 0:
                            nc.vector.tensor_scalar_mul(
                                out=dst_sb[:, t * D:(t + 1) * D],
                                in0=h0_sb[:, t * D:(t + 1) * D], scalar1=EPS)
                            continue
                        gat = sbp.tile([128, sk_max * TW], bf16, tag="gat",
                                       bufs=2)
                        idxc = sbp.tile([128, 8 * sk_max], i16, tag="idxc",
                                        bufs=6)
                        nc.sync.dma_start(
                            out=idxc[:, :8 * sk],
                            in_=idx16[:, 8 * base:8 * (base + sk)])
                        for w in range(NWIN):
                            kw = int(K[t][w])
                            if kw == 0:
                                continue
                            wbase = w * WIN
                            wlen = min(WIN, TROWS - wbase)
                            ga = gat[:, :]
                            for j in range(0, kw, 8):
                                kwc = min(8, kw - j)
                                oj = int(off[t][w]) + j
                                gv = bass.AP(ga.tensor, ga.offset + oj * TW,
                                             [ga.ap[0], (TW, kwc), (1, TW)])
                                nc.gpsimd.dma_gather(
                                    out_ap=gv,
                                    in_ap=tabsrc[wbase:wbase + wlen, :],
                                    idxs_ap=idxc[:, 8 * oj:8 * (oj + kwc)],
                                    num_idxs=128 * kwc,
                                    num_idxs_reg=128 * kwc,
                                    elem_size=TW, queue_num=qrr[0] % 4,
                                    single_packet=True)
                                qrr[0] += 1
                        ga = gat[:, :]
                        alpha = sbp.tile([128, sk_max], f32, tag="alpha",
                                         bufs=4)
                        hlsrc = bass.AP(ga.tensor, ga.offset + D,
                                        [ga.ap[0], (TW, sk)])
                        nc.scalar.activation(
                            alpha[:, :sk], hlsrc,
                            mybir.ActivationFunctionType.Tanh,
                            bias=hr_sb[:, t:t + 1])
                        nc.vector.tensor_mul(
                            out=alpha[:, :sk], in0=alpha[:, :sk],
                            in1=ew_sb[:, base:base + sk])
                        msg = sbp.tile([128, sk_max * D], bf16, tag="msg",
                                       bufs=3)
                        gsrc = bass.AP(ga.tensor, ga.offset,
                                       [ga.ap[0], (TW, sk), (1, D)])
                        m3 = msg[:].rearrange("p (s d) -> p s d", s=sk_max,
                                              d=D)[:, :sk, :]
                        nc.vector.tensor_tensor(
                            out=m3, in0=gsrc,
                            in1=seg_bcast(alpha[:, :sk], sk, D),
                            op=mybir.AluOpType.mult)
                        ma = msg[:, :]
                        mview = bass.AP(ma.tensor, ma.offset,
                                        [ma.ap[0], (1, D), (D, sk)])
                        agg = sbp.tile([128, D], f32, tag="agg", bufs=4)
                        nc.vector.tensor_reduce(
                            out=agg[:], in_=mview, axis=mybir.AxisListType.X,
                            op=mybir.AluOpType.add)
                        eh = sbp.tile([128, D], f32, tag="eh", bufs=4)
                        nc.vector.tensor_scalar_mul(
                            out=eh[:], in0=h0_sb[:, t * D:(t + 1) * D],
                            scalar1=EPS)
                        nc.vector.tensor_tensor(
                            out=dst_sb[:, t * D:(t + 1) * D], in0=agg[:],
                            in1=eh[:], op=mybir.AluOpType.add)

            build_aug(h0_sb, al1_sb, aug_loc1, tab1, ltab1, "aug1")
            layer(ltab1, h0_sb, h1_sb, ar1_sb, "layer1")
            build_aug(h1_sb, al2_sb, aug_loc2, tab2, ltab2, "aug2")
            layer(ltab2, h1_sb, h1_sb, ar2_sb, "layer2")

            with nc.named_scope("t2_softmax"):
                onecol = sbp.tile([128, 1], f32)
                nc.vector.memset(onecol[:], 1.0)
                for t in range(NT):
                    haug = sbp.tile([128, D + 1], f32, tag="haug", bufs=2)
                    nc.vector.tensor_copy(out=haug[:, :D],
                                          in_=h1_sb[:, t * D:(t + 1) * D])
                    nc.vector.tensor_copy(out=haug[:, D:D + 1],
                                          in_=onecol[:])
                    hT = psp.tile([65, 128], f32, tag="hT", bufs=1)
                    nc.tensor.transpose(out=hT[:, :], in_=haug[:, :],
                                        identity=ident[:])
                    hTs = sbp.tile([D + 1, 128], f32, tag="hTs", bufs=2)
                    nc.vector.tensor_copy(out=hTs[:], in_=hT[:, :])
                    lg = psp.tile([128, C], f32, tag="lg", bufs=1)
                    nc.tensor.matmul(lg[:], lhsT=hTs[:], rhs=w2_sb[:, :],
                                     start=True, stop=True)
                    nc.vector.tensor_copy(out=logit_sb[:, t * C:(t + 1) * C],
                                          in_=lg[:])
                l3 = logit_sb[:].rearrange("p (t c) -> p t c", t=NT, c=C)
                mx = sbp.tile([128, NT], f32)
                nc.vector.tensor_reduce(out=mx[:], in_=l3,
                                        axis=mybir.AxisListType.X,
                                        op=mybir.AluOpType.max)
                nc.vector.tensor_tensor(out=l3, in0=l3,
                                        in1=seg_bcast(mx[:, :], NT, C),
                                        op=mybir.AluOpType.subtract)
                ex = sbp.tile([128, NT * C], f32)
                nc.scalar.activation(ex[:], logit_sb[:],
                                     mybir.ActivationFunctionType.Exp)
                sm = sbp.tile([128, NT], f32)
                nc.vector.tensor_reduce(
                    out=sm[:], in_=ex[:].rearrange("p (t c) -> p t c",
                                                   t=NT, c=C),
                    axis=mybir.AxisListType.X, op=mybir.AluOpType.add)
                lsm = sbp.tile([128, NT], f32)
                nc.scalar.activation(lsm[:], sm[:],
                                     mybir.ActivationFunctionType.Ln)
                nc.vector.tensor_tensor(out=l3, in0=l3,
                                        in1=seg_bcast(lsm[:, :], NT, C),
                                        op=mybir.AluOpType.subtract)
                for t in range(NT):
                    nc.sync.dma_start(out=out[t * 128:(t + 1) * 128, :],
                                      in_=logit_sb[:, t * C:(t + 1) * C])
    nc.compile()
    return nc


# ---------------------------------------------------------------- entrypoint
def kernel(x, edge_index, edge_weight_train, t1_w, t1_b,
           att_l1, att_r1, att_l2, att_r2, t2_w, t2_b, _want_trace=False):
    from concourse.bass_utils import run_bass_kernel_spmd
    in_maps, orders, K, off, tbase, totslots = prep(x, edge_index,
                                                    edge_weight_train)
    wm = make_weight_maps(t1_w, t1_b, att_l1, att_r1, att_l2, att_r2,
                          t2_w, t2_b)
    for im in in_maps:
        im.update(wm)
    ncb = build_program(K, off, tbase, totslots)
    kw = dict(trace=True, trace_cores=[0]) if _want_trace else {}
    res = run_bass_kernel_spmd(ncb, in_maps, core_ids=list(range(NC)), **kw)
    out = np.empty((N, C), np.float32)
    for c in range(NC):
        oc = res.results[c]["out"]
        onode = orders[c]
        real = onode < NPC
        out[c * NPC + onode[real]] = oc[real]
    if _want_trace:
        kernel.last_perf = (res.exec_time_ns, res.per_core_scope_times,
                            res.instructions_and_trace[1]
                            if res.instructions_and_trace else None)
    return out



# revision 5
# speedup vs baseline: 2.3622x; 2.3622x over previous
"""FAGCN (2x FAConv) distributed Bass kernel for 8 Trainium2 NeuronCores.

Sharding: 12500 nodes/core (padded 12544); edges dst-sharded. Per layer the
cores AllGather a bf16 table [100352, 128] = [h(64)|hl|pad] and gather source
rows per edge with dma_gather (int16 idx over 4 windows of 25088 rows each,
aligned to source-core pairs so window membership is packing-invariant).
Per-edge coef tanh(hl_src+hr_dst)*sigmoid(ew) and segment sums run on ACT/DVE
in a node-partition supertile layout ([128 nodes x padded slot cols], slots
grouped by window; nodes packed by per-window degree lexsort). t1/t2 matmuls
on TensorE with host-pretransposed weights (bias folded).
"""
import sys
sys.path.insert(0, "/opt/trn_rl_repo")
import numpy as np

# problem constants (hardcoded per harness contract)
N = 100000
E = 1600000
F_IN = 500
D = 64
C = 40
EPS = 0.3
NC = 8
NPC = N // NC            # 12500
NT = 98                  # supertiles per core
NPAD = NT * 128          # 12544
TROWS = NC * NPAD        # 100352 table rows
WINB = 2 * NPAD          # 25088-row windows = 2 source-core blocks
NWIN = 4
FA = 504                 # padded t1 contraction (500 feat + bias + pad)
KC = 126                 # contraction chunk (4 x 126)
TW = 128                 # bf16 table row width (256B)
KWCAP = 8                # max slot cols per gather call (64 desc/engine packet cap)


# ---------------------------------------------------------------- host prep
def _pack(dw):
    # group nodes by dominant window, then peak/total degree desc; then
    # order supertiles by K-profile so harmonized maxima across cores stay
    # close to per-core maxima.
    am = dw.argmax(1)
    o = np.lexsort((-dw.sum(1), -dw.max(1), am))
    Kc = dw[o].reshape(NT, 128, NWIN).max(axis=1)
    pt = np.lexsort(tuple(-Kc[:, w] for w in range(NWIN - 1, -1, -1)))
    return o.reshape(NT, 128)[pt].reshape(-1)


def prep(x, edge_index, edge_weight_train):
    src = np.asarray(edge_index[0]).astype(np.int64)
    dst = np.asarray(edge_index[1]).astype(np.int64)
    ew = np.asarray(edge_weight_train, np.float32)
    x = np.asarray(x, np.float32)
    core_of_dst = dst // NPC
    # window of a source = src_core // 2: invariant to in-core packing
    swin = (src // NPC) // 2
    # idx within window = (src_core % 2) * NPAD + packed local position
    src_core = src // NPC
    src_loc = src % NPC

    orders, Kcs, percore = [], [], []
    for c in range(NC):
        m = core_of_dst == c
        dw = np.zeros((NPAD, NWIN), np.int64)
        np.add.at(dw, (dst[m] - c * NPC, swin[m]), 1)
        o = _pack(dw)
        orders.append(o)
        inv = np.empty(NPAD, np.int64)
        inv[o] = np.arange(NPAD)
        dpos = inv[dst[m] - c * NPC]
        Kcs.append(dw[o].reshape(NT, 128, NWIN).max(axis=1))
        percore.append((m, dpos))

    # packed local position of every node (for idx values)
    posl = np.empty(N, np.int64)
    for c in range(NC):
        inv = np.empty(NPAD, np.int64)
        inv[orders[c]] = np.arange(NPAD)
        posl[c * NPC:(c + 1) * NPC] = inv[:NPC]
    srel = (src_core % 2) * NPAD + posl[src]   # idx within window [0, 25088)

    K = np.maximum.reduce(np.array(Kcs), axis=0)   # harmonized [NT, NWIN]
    off = np.zeros((NT, NWIN), np.int64)
    for t in range(NT):
        o = 0
        for w in range(NWIN):
            off[t, w] = o
            o += K[t, w]
    skt = K.sum(axis=1)
    tbase = np.concatenate([[0], np.cumsum(skt)]).astype(np.int64)
    totslots = int(tbase[-1])

    in_maps = []
    for c in range(NC):
        m, dpos = percore[c]
        sw = swin[m]
        sr = srel[m]
        ewc = ew[m]
        order = np.lexsort((sr, sw, dpos))
        ds, ws, rs, es = dpos[order], sw[order], sr[order], ewc[order]
        grp = ds * NWIN + ws
        first = np.r_[True, grp[1:] != grp[:-1]]
        idxf = np.arange(len(grp))
        start = np.maximum.accumulate(np.where(first, idxf, 0))
        rank = idxf - start
        tt, pp = ds // 128, ds % 128
        slot = tbase[tt] + off[tt, ws] + rank

        ew_arr = np.zeros((128, totslots), np.float32)
        ew_arr[pp, slot] = es
        flat = np.zeros((totslots, 128), np.int64)
        flat[slot, pp] = rs
        idx_arr = np.zeros((128, 8 * totslots), np.int16)
        for t in range(NT):
            for w in range(NWIN):
                kw = int(K[t, w])
                if kw == 0:
                    continue
                cb = int(tbase[t] + off[t, w])
                fl = flat[cb:cb + kw, :].reshape(-1)       # i = d*128+p
                wrapped = fl.reshape(8 * kw, 16).T          # [16, 8k]
                idx_arr[:, 8 * cb:8 * (cb + kw)] = np.tile(
                    wrapped, (8, 1)).astype(np.int16)

        xp = np.zeros((NPAD, F_IN), np.float32)
        onode = orders[c]
        real = onode < NPC
        xp[real] = x[c * NPC + onode[real]]
        xT = np.zeros((FA, NPAD), np.float32)
        xT[:F_IN] = xp.T
        xT[F_IN] = 1.0
        in_maps.append({"xT": xT, "idx16": idx_arr, "ewv": ew_arr})
    return in_maps, orders, K, off, tbase, totslots


def make_weight_maps(t1_w, t1_b, att_l1, att_r1, att_l2, att_r2, t2_w, t2_b):
    t1f = np.zeros((FA, D), np.float32)
    t1f[:F_IN] = np.asarray(t1_w, np.float32).T
    t1f[F_IN] = np.asarray(t1_b, np.float32)
    t1wT = t1f.reshape(4, KC, D).transpose(1, 0, 2).reshape(KC, 4 * D)
    t2wT = np.zeros((D + 1, C), np.float32)
    t2wT[:D] = np.asarray(t2_w, np.float32).T
    t2wT[D] = np.asarray(t2_b, np.float32)
    rep = lambda v: np.tile(np.asarray(v, np.float32)[None, :], (128, 1))
    return {"t1wT": t1wT, "t2wT": t2wT,
            "attl1": rep(att_l1), "attr1": rep(att_r1),
            "attl2": rep(att_l2), "attr2": rep(att_r2)}


# ---------------------------------------------------------------- device prog
def build_program(K, off, tbase, totslots):
    import concourse.bass as bass
    import concourse.bacc as bacc
    import concourse.mybir as mybir
    import concourse.tile as tile
    from concourse.masks import make_identity
    f32, bf16, i16 = (mybir.dt.float32, mybir.dt.bfloat16, mybir.dt.int16)

    nc = bacc.Bacc("TRN2", target_bir_lowering=False, debug=False,
                   num_devices=NC, num_swdge_queues=4,
                   dynamic_dma_scratch_size=32768)
    xT = nc.dram_tensor("xT", [FA, NPAD], f32, kind="ExternalInput")
    t1wT = nc.dram_tensor("t1wT", [KC, 4 * D], f32, kind="ExternalInput")
    t2wT = nc.dram_tensor("t2wT", [D + 1, C], f32, kind="ExternalInput")
    attl1 = nc.dram_tensor("attl1", [128, D], f32, kind="ExternalInput")
    attr1 = nc.dram_tensor("attr1", [128, D], f32, kind="ExternalInput")
    attl2 = nc.dram_tensor("attl2", [128, D], f32, kind="ExternalInput")
    attr2 = nc.dram_tensor("attr2", [128, D], f32, kind="ExternalInput")
    idx16 = nc.dram_tensor("idx16", [128, 8 * totslots], i16,
                           kind="ExternalInput")
    ewv = nc.dram_tensor("ewv", [128, totslots], f32, kind="ExternalInput")
    out = nc.dram_tensor("out", [NPAD, C], f32, kind="ExternalOutput")

    sk_max = int(max(sum(K[t]) for t in range(NT)))

    def seg_bcast(ap2d, segs, width):
        # [128, segs] AP -> [128, segs, width] AP (inner broadcast)
        return bass.AP(ap2d.tensor, ap2d.offset,
                       [ap2d.ap[0], (ap2d.ap[-1][0], segs), (0, width)])

    def rep_mid(ap2d, segs, width):
        # [128, width] AP -> [128, segs, width] AP (middle repeat)
        return bass.AP(ap2d.tensor, ap2d.offset,
                       [ap2d.ap[0], (0, segs), (1, width)])

    with tile.TileContext(nc) as tc:
        with tc.tile_pool(name="sbuf", bufs=1) as sbp, \
             tc.tile_pool(name="dram", bufs=1, space="DRAM") as dram, \
             tc.tile_pool(name="psum", bufs=1, space="PSUM") as psp:

            h0_sb = sbp.tile([128, NT * D], f32)
            h1_sb = sbp.tile([128, NT * D], f32)
            hl_sb = sbp.tile([128, NT], f32)
            hr_sb = sbp.tile([128, NT], f32)
            ew_sb = sbp.tile([128, totslots], f32)
            w1_sb = sbp.tile([KC, 4 * D], f32)
            w2_sb = sbp.tile([D + 1, C], f32)
            al1_sb = sbp.tile([128, D], f32)
            ar1_sb = sbp.tile([128, D], f32)
            al2_sb = sbp.tile([128, D], f32)
            ar2_sb = sbp.tile([128, D], f32)
            ident = sbp.tile([128, 128], f32)

            tab1 = dram.tile([TROWS, TW], bf16, addr_space="Shared",
                             name="tab1")
            tab2 = dram.tile([TROWS, TW], bf16, addr_space="Shared",
                             name="tab2")
            ltab1 = dram.tile([TROWS, TW], bf16, name="ltab1")
            ltab2 = dram.tile([TROWS, TW], bf16, name="ltab2")
            aug_loc1 = dram.tile([NPAD, TW], bf16, name="aug_loc1")
            aug_loc2 = dram.tile([NPAD, TW], bf16, name="aug_loc2")

            make_identity(nc, ident[:])
            nc.sync.dma_start(out=w1_sb[:], in_=t1wT[:, :])
            nc.sync.dma_start(out=w2_sb[:], in_=t2wT[:, :])
            nc.sync.dma_start(out=al1_sb[:], in_=attl1[:, :])
            nc.sync.dma_start(out=ar1_sb[:], in_=attr1[:, :])
            nc.sync.dma_start(out=al2_sb[:], in_=attl2[:, :])
            nc.sync.dma_start(out=ar2_sb[:], in_=attr2[:, :])

            with nc.named_scope("ew_prep"):
                ewr = sbp.tile([128, totslots], f32, tag="ewr", bufs=1)
                nc.sync.dma_start(out=ewr[:], in_=ewv[:, :])
                nc.scalar.activation(ew_sb[:], ewr[:],
                                     mybir.ActivationFunctionType.Sigmoid)
                nc.vector.tensor_scalar(out=ewr[:], in0=ewr[:], scalar1=0.0,
                                        scalar2=None,
                                        op0=mybir.AluOpType.not_equal)
                nc.vector.tensor_mul(out=ew_sb[:], in0=ew_sb[:], in1=ewr[:])

            # phase 1: h0 = relu(x @ t1_w.T + b)
            with nc.named_scope("t1_matmul"):
                CH = 4
                for blk in range((NT + CH - 1) // CH):
                    n0 = blk * CH * 128
                    nch = min(CH, NT - blk * CH)
                    pt = [psp.tile([128, D], f32, tag=f"p1_{t}", bufs=1,
                                   name=f"p1_{blk}_{t}") for t in range(nch)]
                    for cc in range(4):
                        xc = sbp.tile([KC, CH * 128], f32, tag="xstage",
                                      bufs=2)
                        nc.sync.dma_start(
                            out=xc[:, :nch * 128],
                            in_=xT[cc * KC:(cc + 1) * KC, n0:n0 + nch * 128])
                        for t in range(nch):
                            nc.tensor.matmul(
                                pt[t][:], lhsT=xc[:, t * 128:(t + 1) * 128],
                                rhs=w1_sb[:, cc * D:(cc + 1) * D],
                                start=(cc == 0), stop=(cc == 3))
                    for t in range(nch):
                        gt = blk * CH + t
                        nc.scalar.activation(
                            h0_sb[:, gt * D:(gt + 1) * D], pt[t][:],
                            mybir.ActivationFunctionType.Relu)

            def att_dots(h_sb, att_sb, out_col):
                CHT = 14
                for b in range(NT // CHT):
                    tmp = sbp.tile([128, CHT * D], f32, tag="adtmp", bufs=2)
                    sl = h_sb[:, b * CHT * D:(b + 1) * CHT * D]
                    nc.vector.tensor_tensor(
                        out=tmp[:], in0=sl, in1=rep_mid(att_sb[:, :], CHT, D),
                        op=mybir.AluOpType.mult)
                    nc.vector.tensor_reduce(
                        out=out_col[:, b * CHT:(b + 1) * CHT],
                        in_=tmp[:].rearrange("p (t d) -> p t d", t=CHT, d=D),
                        axis=mybir.AxisListType.X, op=mybir.AluOpType.add)

            def build_aug(h_sb, al_sb, aug_loc, tabdst, ltabdst, scope):
                with nc.named_scope(scope):
                    att_dots(h_sb, al_sb, hl_sb)
                    for t in range(NT):
                        aug = sbp.tile([128, TW], bf16, tag="augt", bufs=3)
                        nc.vector.tensor_copy(
                            out=aug[:, :D], in_=h_sb[:, t * D:(t + 1) * D])
                        nc.vector.tensor_copy(out=aug[:, D:D + 1],
                                              in_=hl_sb[:, t:t + 1])
                        nc.sync.dma_start(
                            out=aug_loc[t * 128:(t + 1) * 128, :],
                            in_=aug[:])
                    nc.gpsimd.collective_compute(
                        "AllGather", mybir.AluOpType.bypass,
                        replica_groups=[list(range(NC))],
                        ins=[aug_loc[:, :]], outs=[tabdst[:, :]])
                    nc.sync.dma_start(out=ltabdst[:, :], in_=tabdst[:, :])

            qrr = [0]

            def layer(tabsrc, src_sb, dst_sb, ar_sb, scope):
                with nc.named_scope(scope):
                    att_dots(src_sb, ar_sb, hr_sb)
                    for t in range(NT):
                        sk = int(sum(K[t]))
                        base = int(tbase[t])
                        if sk == 0:
                            nc.vector.tensor_scalar_mul(
                                out=dst_sb[:, t * D:(t + 1) * D],
                                in0=h0_sb[:, t * D:(t + 1) * D], scalar1=EPS)
                            continue
                        gat = sbp.tile([128, sk_max * TW], bf16, tag="gat",
                                       bufs=3)
                        idxc = sbp.tile([128, 8 * sk_max], i16, tag="idxc",
                                        bufs=6)
                        nc.sync.dma_start(
                            out=idxc[:, :8 * sk],
                            in_=idx16[:, 8 * base:8 * (base + sk)])
                        for w in range(NWIN):
                            kw = int(K[t][w])
                            if kw == 0:
                                continue
                            wbase = w * WINB
                            ga = gat[:, :]
                            for j in range(0, kw, KWCAP):
                                kwc = min(KWCAP, kw - j)
                                oj = int(off[t][w]) + j
                                gv = bass.AP(ga.tensor, ga.offset + oj * TW,
                                             [ga.ap[0], (TW, kwc), (1, TW)])
                                nc.gpsimd.dma_gather(
                                    out_ap=gv,
                                    in_ap=tabsrc[wbase:wbase + WINB, :],
                                    idxs_ap=idxc[:, 8 * oj:8 * (oj + kwc)],
                                    num_idxs=128 * kwc,
                                    num_idxs_reg=128 * kwc,
                                    elem_size=TW, queue_num=qrr[0] % 4,
                                    single_packet=True)
                                qrr[0] += 1
                        ga = gat[:, :]
                        alpha = sbp.tile([128, sk_max], f32, tag="alpha",
                                         bufs=4)
                        hlsrc = bass.AP(ga.tensor, ga.offset + D,
                                        [ga.ap[0], (TW, sk)])
                        nc.scalar.activation(
                            alpha[:, :sk], hlsrc,
                            mybir.ActivationFunctionType.Tanh,
                            bias=hr_sb[:, t:t + 1])
                        nc.vector.tensor_mul(
                            out=alpha[:, :sk], in0=alpha[:, :sk],
                            in1=ew_sb[:, base:base + sk])
                        msg = sbp.tile([128, sk_max * D], bf16, tag="msg",
                                       bufs=3)
                        gsrc = bass.AP(ga.tensor, ga.offset,
                                       [ga.ap[0], (TW, sk), (1, D)])
                        m3 = msg[:].rearrange("p (s d) -> p s d", s=sk_max,
                                              d=D)[:, :sk, :]
                        nc.vector.tensor_tensor(
                            out=m3, in0=gsrc,
                            in1=seg_bcast(alpha[:, :sk], sk, D),
                            op=mybir.AluOpType.mult)
                        ma = msg[:, :]
                        mview = bass.AP(ma.tensor, ma.offset,
                                        [ma.ap[0], (1, D), (D, sk)])
                        agg = sbp.tile([128, D], f32, tag="agg", bufs=4)
                        nc.vector.tensor_reduce(
                            out=agg[:], in_=mview, axis=mybir.AxisListType.X,
                            op=mybir.AluOpType.add)
                        eh = sbp.tile([128, D], f32, tag="eh", bufs=4)
                        nc.vector.tensor_scalar_mul(
                            out=eh[:], in0=h0_sb[:, t * D:(t + 1) * D],
                            scalar1=EPS)
                        nc.vector.tensor_tensor(
                            out=dst_sb[:, t * D:(t + 1) * D], in0=agg[:],
                            in1=eh[:], op=mybir.AluOpType.add)

            build_aug(h0_sb, al1_sb, aug_loc1, tab1, ltab1, "aug1")
            layer(ltab1, h0_sb, h1_sb, ar1_sb, "layer1")
            build_aug(h1_sb, al2_sb, aug_loc2, tab2, ltab2, "aug2")
            layer(ltab2, h1_sb, h1_sb, ar2_sb, "layer2")

            with nc.named_scope("t2_softmax"):
                onecol = sbp.tile([128, 1], f32)
                nc.vector.memset(onecol[:], 1.0)
                for t in range(NT):
                    haug = sbp.tile([128, D + 1], f32, tag="haug", bufs=2)
                    nc.vector.tensor_copy(out=haug[:, :D],
                                          in_=h1_sb[:, t * D:(t + 1) * D])
                    nc.vector.tensor_copy(out=haug[:, D:D + 1],
                                          in_=onecol[:])
                    hT = psp.tile([65, 128], f32, tag="hT", bufs=1)
                    nc.tensor.transpose(out=hT[:, :], in_=haug[:, :],
                                        identity=ident[:])
                    hTs = sbp.tile([D + 1, 128], f32, tag="hTs", bufs=2)
                    nc.vector.tensor_copy(out=hTs[:], in_=hT[:, :])
                    lg = psp.tile([128, C], f32, tag="lg", bufs=1)
                    nc.tensor.matmul(lg[:], lhsT=hTs[:], rhs=w2_sb[:, :],
                                     start=True, stop=True)
                    lgs = sbp.tile([128, C], f32, tag="lgs", bufs=2)
                    nc.vector.tensor_copy(out=lgs[:], in_=lg[:])
                    mx = sbp.tile([128, 1], f32, tag="mx", bufs=2)
                    nc.vector.tensor_reduce(out=mx[:], in_=lgs[:],
                                            axis=mybir.AxisListType.X,
                                            op=mybir.AluOpType.max)
                    nc.vector.tensor_tensor(out=lgs[:], in0=lgs[:],
                                            in1=seg_bcast(mx[:, :], 1, C),
                                            op=mybir.AluOpType.subtract)
                    ex = sbp.tile([128, C], f32, tag="ex", bufs=2)
                    nc.scalar.activation(ex[:], lgs[:],
                                         mybir.ActivationFunctionType.Exp)
                    sm = sbp.tile([128, 1], f32, tag="sm", bufs=2)
                    nc.vector.tensor_reduce(out=sm[:], in_=ex[:],
                                            axis=mybir.AxisListType.X,
                                            op=mybir.AluOpType.add)
                    lsm = sbp.tile([128, 1], f32, tag="lsm", bufs=2)
                    nc.scalar.activation(lsm[:], sm[:],
                                         mybir.ActivationFunctionType.Ln)
                    nc.vector.tensor_tensor(out=lgs[:], in0=lgs[:],
                                            in1=seg_bcast(lsm[:, :], 1, C),
                                            op=mybir.AluOpType.subtract)
                    nc.sync.dma_start(out=out[t * 128:(t + 1) * 128, :],
                                      in_=lgs[:])
    nc.compile()
    return nc


# ---------------------------------------------------------------- entrypoint
def kernel(x, edge_index, edge_weight_train, t1_w, t1_b,
           att_l1, att_r1, att_l2, att_r2, t2_w, t2_b, _want_trace=False):
    from concourse.bass_utils import run_bass_kernel_spmd
    in_maps, orders, K, off, tbase, totslots = prep(x, edge_index,
                                                    edge_weight_train)
    wm = make_weight_maps(t1_w, t1_b, att_l1, att_r1, att_l2, att_r2,
                          t2_w, t2_b)
    for im in in_maps:
        im.update(wm)
    ncb = build_program(K, off, tbase, totslots)
    kw = dict(trace=True, trace_cores=[0]) if _want_trace else {}
    res = run_bass_kernel_spmd(ncb, in_maps, core_ids=list(range(NC)), **kw)
    out = np.empty((N, C), np.float32)
    for c in range(NC):
        oc = res.results[c]["out"]
        onode = orders[c]
        real = onode < NPC
        out[c * NPC + onode[real]] = oc[real]
    if _want_trace:
        kernel.last_perf = (res.exec_time_ns, res.per_core_scope_times,
                            res.instructions_and_trace[1]
                            if res.instructions_and_trace else None)
    return out
